# revision 7
# baseline (speedup 1.0000x reference)
"""ALiBi multi-head causal attention on 8 Trainium2 NeuronCores.

Problem: x[4,2048,1024] @ w_qkv[1024,3072] -> 16-head causal attention with
ALiBi bias -> @ w_o[1024,1024].

Sharding: 8 cores = 4 batches x 2 head-groups (8 heads each). Each core
computes its batch's attention for its 8 heads plus the partial output
projection over its 512 rows of w_o; the host sums the two partials per batch.

Per-core pipeline (matmuls in float32r = fp32 storage with 11-bit RNE input
rounding, full PE rate at free-dim >= 256):
  1. QKV projection. qT,kT produced in [dh, T] layout (head pairs packed into
     128-partition tiles), v in [T, dh] layout - no transposes anywhere.
  2. Scores as sT[k,q] tiles: a K=64 main matmul plus a K=3 "augmentation"
     matmul accumulating into the same PSUM bank. The aug rows carry a hi/lo
     split of slope*k (exact to fp32 despite fp32r rounding) and -slope*q on
     the q side (whose rounding error is constant per softmax row and cancels
     in normalization). Head pairs run concurrently in the PE array via row
     tiling. The 1/sqrt(C) scale is baked into the Q weights on the host.
  3. Causal mask added on diagonal tiles only; exp on the scalar engine
     straight from PSUM over [128,1024] two-head strips.
  4. P@V with an all-ones row appended to V (M=65) so the softmax
     denominators fall out of the same matmuls for free.
  5. Denominators replicated across partitions with a tiny K=2 matmul,
     reciprocal, normalization fused into the PSUM->SBUF copy, and the
     output projection runs per q-tile slab.
"""
import sys
sys.path.insert(0, '/opt/trn_rl_repo')

import numpy as np

B, T, D = 4, 2048, 1024
H = 16
HG = 8            # heads per core (head-group)
NCORES = 8
NEG = -1.0e30

_cached = {}


def _slopes():
    x = (2.0 ** 8) ** (1.0 / H)
    return 1.0 / x ** np.arange(1, H + 1, dtype=np.float64)


def _rne11(v):
    """Round f32 array to 11 mantissa bits (fp32r's input rounding)."""
    m, e = np.frexp(v.astype(np.float64))
    q = np.round(m * (1 << 12)) / (1 << 12)
    return np.ldexp(q, e).astype(np.float32)


def _build_module():
    import concourse.bass as bass
    import concourse.mybir as mybir
    import concourse.tile as tile
    from concourse import bacc
    from contextlib import ExitStack

    f32, f32r, bf16 = mybir.dt.float32, mybir.dt.float32r, mybir.dt.bfloat16
    ADD = mybir.AluOpType.add
    MULT = mybir.AluOpType.mult
    EXP = mybir.ActivationFunctionType.Exp

    nc = bacc.Bacc("TRN2", target_bir_lowering=False, debug=False)

    xt_d = nc.dram_tensor("xt", [D, T], f32, kind="ExternalInput")
    wq_d = nc.dram_tensor("wq", [D, 512], f32, kind="ExternalInput")
    wk_d = nc.dram_tensor("wk", [D, 512], f32, kind="ExternalInput")
    wv_d = nc.dram_tensor("wv", [D, 512], f32, kind="ExternalInput")
    wo_d = nc.dram_tensor("wo", [512, D], f32, kind="ExternalInput")
    augk_d = nc.dram_tensor("augk", [2, 128, T], f32, kind="ExternalInput")
    augq_d = nc.dram_tensor("augq", [2, 128, T], f32, kind="ExternalInput")
    masks_d = nc.dram_tensor("masks", [128, 4096], bf16, kind="ExternalInput")
    emat_d = nc.dram_tensor("emat", [2, 128], f32, kind="ExternalInput")
    out_d = nc.dram_tensor("out", [T, D], f32, kind="ExternalOutput")

    NTB = T // 128            # 16 t-blocks
    NQJ = T // 512            # 4 q-tiles

    with tile.TileContext(nc) as tc:
        with ExitStack() as ctx:
            pers = ctx.enter_context(tc.tile_pool(name="pers", bufs=1))

            qd = [pers.tile([128, T], f32r, name=f"qd{p}", tag=f"qd{p}")
                  for p in range(4)]
            kd = [pers.tile([128, T], f32r, name=f"kd{p}", tag=f"kd{p}")
                  for p in range(4)]
            augq = [pers.tile([128, T], f32r, name=f"augq{g}", tag=f"augq{g}")
                    for g in range(2)]
            augk = [pers.tile([128, T], f32r, name=f"augk{g}", tag=f"augk{g}")
                    for g in range(2)]
            vt = pers.tile([128, NTB, HG, 65], f32r, name="vt")
            masks_sb = pers.tile([128, 4, 1024], bf16, name="masks_sb")
            emat_sb = pers.tile([2, 128], f32r, name="emat_sb")

            nc.sync.dma_start(masks_sb[:], masks_d.ap().rearrange(
                "p (m q) -> p m q", m=4))
            nc.sync.dma_start(emat_sb[:], emat_d.ap().bitcast(f32r))
            for g in range(2):
                nc.sync.dma_start(augk[g][:], augk_d.ap()[g].bitcast(f32r))
                nc.sync.dma_start(augq[g][:], augq_d.ap()[g].bitcast(f32r))

            xt_r = xt_d.ap().rearrange("(cb p) t -> p cb t", p=128).bitcast(f32r)

            # ---- Phase 1a: Q/K projection ----
            with tc.tile_pool(name="projw", bufs=1) as projw, \
                 tc.tile_pool(name="projx", bufs=2) as projx, \
                 tc.tile_pool(name="projps", bufs=2, space="PSUM") as projps:
                wq_sb = projw.tile([128, 8, 512], f32r, name="wq_sb")
                wk_sb = projw.tile([128, 8, 512], f32r, name="wk_sb")
                nc.sync.dma_start(
                    wq_sb[:], wq_d.ap().rearrange("(cb p) d -> p cb d", p=128).bitcast(f32r))
                nc.sync.dma_start(
                    wk_sb[:], wk_d.ap().rearrange("(cb p) d -> p cb d", p=128).bitcast(f32r))
                for tt in range(T // 256):
                    xt_t = projx.tile([128, 8, 256], f32r, tag="xt_t")
                    nc.sync.dma_start(xt_t[:], xt_r[:, :, tt * 256:(tt + 1) * 256])
                    tsl = slice(tt * 256, (tt + 1) * 256)
                    for ob in range(4):
                        psq = projps.tile([128, 256], f32, tag="psq")
                        for cb in range(8):
                            nc.tensor.matmul(
                                psq[:], wq_sb[:, cb, ob * 128:(ob + 1) * 128],
                                xt_t[:, cb, :], start=cb == 0, stop=cb == 7)
                        nc.vector.tensor_copy(qd[ob][:, tsl], psq[:])
                        psk = projps.tile([128, 256], f32, tag="psk")
                        for cb in range(8):
                            nc.tensor.matmul(
                                psk[:], wk_sb[:, cb, ob * 128:(ob + 1) * 128],
                                xt_t[:, cb, :], start=cb == 0, stop=cb == 7)
                        nc.scalar.copy(kd[ob][:, tsl], psk[:])

            # ---- Phase 1b: V projection ----
            with tc.tile_pool(name="projwv", bufs=1) as projwv, \
                 tc.tile_pool(name="projxv", bufs=2) as projxv, \
                 tc.tile_pool(name="projpsv", bufs=2, space="PSUM") as projpsv:
                wv_sb = projwv.tile([128, 8, 512], f32r, name="wv_sb")
                nc.sync.dma_start(
                    wv_sb[:], wv_d.ap().rearrange("(cb p) d -> p cb d", p=128).bitcast(f32r))
                for tt in range(T // 256):
                    xt_t2 = projxv.tile([128, 8, 256], f32r, tag="xt_t2")
                    nc.sync.dma_start(xt_t2[:], xt_r[:, :, tt * 256:(tt + 1) * 256])
                    for tb2 in range(2):
                        tb = tt * 2 + tb2
                        psv = projpsv.tile([128, 512], f32, tag="psv")
                        for cb in range(8):
                            nc.tensor.matmul(
                                psv[:], xt_t2[:, cb, tb2 * 128:(tb2 + 1) * 128],
                                wv_sb[:, cb, :], start=cb == 0, stop=cb == 7)
                        nc.vector.tensor_copy(
                            vt[:, tb, :, 0:64],
                            psv[:].rearrange("p (j d) -> p j d", j=HG))
                onesf = projxv.tile([128, 1], f32, tag="onesf")
                nc.any.memset(onesf[:], 1.0)
                nc.vector.tensor_copy(
                    vt[:, :, :, 64:65],
                    onesf[:, :, None, None].to_broadcast((128, NTB, HG, 1)))

            # ---- Phase 2: attention + normalize + output projection ----
            with tc.tile_pool(name="attn", bufs=1) as attn, \
                 tc.tile_pool(name="pst", bufs=2) as pst, \
                 tc.tile_pool(name="slabp", bufs=2) as slabp, \
                 tc.tile_pool(name="outp", bufs=2) as outp, \
                 tc.tile_pool(name="scps", bufs=2, space="PSUM") as scps, \
                 tc.tile_pool(name="pvps", bufs=1, space="PSUM") as pvps, \
                 tc.tile_pool(name="repps", bufs=1, space="PSUM") as repps:
                wo_sb = attn.tile([128, 4, D], f32r, name="wo_sb")
                nc.sync.dma_start(
                    wo_sb[:], wo_d.ap().rearrange("(db p) e -> p db e", p=128).bitcast(f32r))

                for qj in range(NQJ):
                    qsl = slice(qj * 512, (qj + 1) * 512)
                    nkb = 4 * (qj + 1)
                    slab = slabp.tile([128, 4, 512], f32r, tag="slab")
                    for pr in range(4):
                        h0, h1 = 2 * pr, 2 * pr + 1
                        g2 = pr // 2
                        a0, a1 = (h0 % 4) * 32, (h1 % 4) * 32
                        pv0 = pvps.tile([65, 512], f32, tag="pv0")
                        pv1 = pvps.tile([65, 512], f32, tag="pv1")
                        for kb in range(nkb):
                            ksl = slice(kb * 128, (kb + 1) * 128)
                            sc = scps.tile([128, 1024], f32, tag="sc")
                            nc.tensor.matmul(
                                sc[:, 0:512], kd[pr][0:64, ksl], qd[pr][0:64, qsl],
                                start=True, stop=False, tile_position=(0, 0))
                            nc.tensor.matmul(
                                sc[:, 0:512], augk[g2][a0:a0 + 3, ksl],
                                augq[g2][a0:a0 + 3, qsl],
                                start=False, stop=True, tile_position=(a0, 0))
                            nc.tensor.matmul(
                                sc[:, 512:1024], kd[pr][64:128, ksl], qd[pr][64:128, qsl],
                                start=True, stop=False, tile_position=(64, 0))
                            nc.tensor.matmul(
                                sc[:, 512:1024], augk[g2][a1:a1 + 3, ksl],
                                augq[g2][a1:a1 + 3, qsl],
                                start=False, stop=True, tile_position=(a1, 0))
                            if kb // 4 == qj:
                                m = kb % 4
                                nc.vector.tensor_tensor(
                                    sc[:], sc[:], masks_sb[:, m, :], ADD)
                            pstrip = pst.tile([128, 1024], f32r, tag="pstrip")
                            nc.scalar.activation(pstrip[:], sc[:], EXP)
                            nc.tensor.matmul(
                                pv0[:], vt[:, kb, h0, :], pstrip[:, 0:512],
                                start=kb == 0, stop=kb == nkb - 1)
                            nc.tensor.matmul(
                                pv1[:], vt[:, kb, h1, :], pstrip[:, 512:1024],
                                start=kb == 0, stop=kb == nkb - 1)
                        spack = pst.tile([2, 512], f32r, tag="spack")
                        nc.vector.tensor_copy(spack[0:1, :], pv0[64:65, :])
                        s1 = pst.tile([1, 512], f32r, tag="s1")
                        nc.vector.tensor_copy(s1[:], pv1[64:65, :])
                        nc.sync.dma_start(spack[1:2, :], s1[:])
                        rep = repps.tile([128, 512], f32, tag="rep")
                        nc.tensor.matmul(rep[:], emat_sb[:], spack[:],
                                         start=True, stop=True)
                        rec = pst.tile([128, 512], f32, tag="rec")
                        nc.vector.reciprocal(rec[:], rep[:])
                        nc.vector.tensor_tensor(
                            slab[0:64, pr, :], pv0[0:64, :], rec[0:64, :], MULT)
                        nc.vector.tensor_tensor(
                            slab[64:128, pr, :], pv1[0:64, :], rec[64:128, :], MULT)

                    # output projection for this q-tile slab (reuses sc slots)
                    for tb_in in range(4):
                        for et in range(2):
                            wo_ps = scps.tile([128, 1024], f32, tag="sc",
                                              name="wo_ps")[:, 0:512]
                            for db in range(4):
                                nc.tensor.matmul(
                                    wo_ps,
                                    slab[:, db, tb_in * 128:(tb_in + 1) * 128],
                                    wo_sb[:, db, et * 512:(et + 1) * 512],
                                    start=db == 0, stop=db == 3)
                            ot = outp.tile([128, 512], f32, tag="ot")
                            nc.vector.tensor_copy(ot[:], wo_ps)
                            r0 = qj * 512 + tb_in * 128
                            nc.sync.dma_start(
                                out_d.ap()[r0:r0 + 128,
                                           et * 512:(et + 1) * 512], ot[:])

    nc.compile()
    return nc


def _host_inputs(x, w_qkv, w_o):
    """Build the 8 per-core input maps from the full problem inputs."""
    import ml_dtypes
    slopes = _slopes()
    scale = float(D) ** 0.5
    k_idx = np.arange(T, dtype=np.float64)

    # mask2[i, m, q'] for two-head strips: same mask in both 512 halves
    mask1 = np.empty((128, 4, 512), np.float32)
    i = np.arange(128)[:, None]
    jq = np.arange(512)[None, :]
    for m in range(4):
        mask1[:, m, :] = np.where(i + 128 * m <= jq, 0.0, NEG)
    mask2 = np.concatenate([mask1, mask1], axis=2)          # [128, 4, 1024]
    masks = mask2.reshape(128, 4096).astype(ml_dtypes.bfloat16)

    emat = np.zeros((2, 128), np.float32)
    emat[0, 0:64] = 1.0
    emat[1, 64:128] = 1.0

    in_maps = []
    for c in range(NCORES):
        b, g = c // 2, c % 2
        hsl = slice(g * 512, (g + 1) * 512)
        xt = np.ascontiguousarray(x[b].T).astype(np.float32)
        wq = (w_qkv[:, :D][:, hsl] / np.float32(scale)).astype(np.float32)
        wk = np.ascontiguousarray(w_qkv[:, D:2 * D][:, hsl]).astype(np.float32)
        wv = np.ascontiguousarray(w_qkv[:, 2 * D:][:, hsl]).astype(np.float32)
        wo = np.ascontiguousarray(w_o[hsl, :]).astype(np.float32)

        augk = np.zeros((2, 128, T), np.float32)
        augq = np.zeros((2, 128, T), np.float32)
        for j in range(HG):
            s = slopes[g * HG + j]
            g2, a = j // 4, (j % 4) * 32
            bias_k = (s * k_idx).astype(np.float32)
            hi = _rne11(bias_k)
            augk[g2, a + 0] = hi
            augk[g2, a + 1] = bias_k - hi
            augk[g2, a + 2] = 1.0
            augq[g2, a + 0] = 1.0
            augq[g2, a + 1] = 1.0
            augq[g2, a + 2] = (-s * k_idx).astype(np.float32)
        in_maps.append({
            "xt": xt, "wq": wq, "wk": wk, "wv": wv, "wo": wo,
            "augk": augk, "augq": augq, "masks": masks, "emat": emat,
        })
    return in_maps


class _Runner:
    """Compile once, execute many times on cores 0-7 via PJRT/axon."""

    def __init__(self, nc):
        import jax
        from jax.sharding import Mesh, PartitionSpec
        from jax.experimental.shard_map import shard_map
        from concourse import bass2jax, mybir
        from concourse.bass2jax import _bass_exec_p, install_neuronx_cc_hook
        install_neuronx_cc_hook()
        self.jax = jax
        self.nc = nc
        pname = nc.partition_id_tensor.name if nc.partition_id_tensor else None
        in_names, out_names, out_avals = [], [], []
        for alloc in nc.m.functions[0].allocations:
            if not isinstance(alloc, mybir.MemoryLocationSet):
                continue
            name = alloc.memorylocations[0].name
            if alloc.kind == "ExternalInput":
                if name != pname:
                    in_names.append(name)
            elif alloc.kind == "ExternalOutput":
                out_names.append(name)
                out_avals.append(jax.core.ShapedArray(
                    tuple(alloc.tensor_shape), mybir.dt.np(alloc.dtype)))
        self.in_names, self.out_names, self.out_avals = in_names, out_names, out_avals
        n_params = len(in_names)
        all_in = in_names + out_names + ([pname] if pname else [])

        def _body(*args):
            ops = list(args)
            if pname:
                ops.append(bass2jax.partition_id_tensor())
            return tuple(_bass_exec_p.bind(
                *ops, out_avals=tuple(out_avals), in_names=tuple(all_in),
                out_names=tuple(out_names), lowering_input_output_aliases=(),
                sim_require_finite=False, sim_require_nnan=False, nc=nc))

        devices = jax.devices()[:NCORES]
        self.mesh = Mesh(np.asarray(devices), ("core",))
        specs = (PartitionSpec("core"),) * (n_params + len(out_names))
        self.fn = jax.jit(
            shard_map(_body, mesh=self.mesh, in_specs=specs,
                      out_specs=(PartitionSpec("core"),) * len(out_names),
                      check_rep=False),
            keep_unused=True)

    def put_inputs(self, in_maps):
        import jax
        from jax.sharding import PartitionSpec
        sh = jax.sharding.NamedSharding(self.mesh, PartitionSpec("core"))
        args = []
        for name in self.in_names:
            cat = np.concatenate([np.asarray(in_maps[c][name])
                                  for c in range(NCORES)], axis=0)
            args.append(jax.device_put(cat, sh))
        for av in self.out_avals:
            z = np.zeros((NCORES * av.shape[0], *av.shape[1:]), av.dtype)
            args.append(jax.device_put(z, sh))
        return args

    def run(self, args):
        outs = self.fn(*args)
        self.jax.block_until_ready(outs)
        return outs

    def results(self, outs):
        return [
            {n: np.asarray(outs[i]).reshape(NCORES, *self.out_avals[i].shape)[c]
             for i, n in enumerate(self.out_names)}
            for c in range(NCORES)
        ]


def _get_runner():
    if "runner" not in _cached:
        _cached["runner"] = _Runner(_build_module())
    return _cached["runner"]


def kernel(x, w_qkv, w_o):
    x = np.asarray(x)
    w_qkv = np.asarray(w_qkv)
    w_o = np.asarray(w_o)
    r = _get_runner()
    args = r.put_inputs(_host_inputs(x, w_qkv, w_o))
    res = r.results(r.run(args))
    out = np.empty((B, T, D), np.float32)
    for b in range(B):
        out[b] = res[2 * b]["out"] + res[2 * b + 1]["out"]
    return out
